# revision 17
# baseline (speedup 1.0000x reference)
"""Trainium2 Bass kernel for nn_ABCLayer (binary-basis conv layer).

Math reduction (conv is linear in its input):
    reference out = sum_n beta_n * (conv(A_n, W_eff) + sum_alpha*bias_n)
                  = conv(sum_n beta_n * A_n, W_eff) + sum_alpha * dot(beta, bias)
with A_n = sign(clip(X+v_n,0,1)-0.5) = sign(X - t_n),  t_n = 0.5 - v_n.

So the device computes ONE elementwise 3-threshold step function
    A(x) = 2*b0*[x>t0] + b1*sign(x-t1) + 2*b2*[x>t2] - (b0+b2)
followed by ONE 3x3 SAME conv (9 accumulating matmuls per output tile) and a
constant add.  W_eff / alpha (5x5 least squares on sign bases) are tiny and
folded on the host.

Distribution: pure data parallel over batch (32 images / 8 cores = 4 each).
The host shards X and pre-transposes each shard to channel-major [128, pix]
so the device needs no transposes at all; per-core outputs come back
channel-major and are un-transposed during the gather.
"""

import sys

import numpy as np

sys.path.insert(0, "/opt/trn_rl_repo")

import ml_dtypes  # noqa: E402
import concourse.bass as bass  # noqa: E402
import concourse.tile as tile  # noqa: E402
from concourse import bacc, mybir  # noqa: E402
from concourse._compat import with_exitstack  # noqa: E402
from concourse.bass_utils import run_bass_kernel_spmd  # noqa: E402

# ---------------------------------------------------------------- geometry
NCORES = 8
NB, H, WID, C = 32, 56, 56, 128        # full input NHWC
NPER = NB // NCORES                    # images per core
PIX = H * WID                            # 3136
RP, CP = H + 2, WID + 4                  # padded activation plane 58 x 60
                                       # (col pad 2 keeps bf16 writes 4B-aligned)
GR = 8                                 # output rows per PSUM group
NGRP = H // GR                         # 7 groups of 8 rows
M_FILTERS = 5

AOT = mybir.AluOpType
AFT = mybir.ActivationFunctionType
F32 = mybir.dt.float32
BF16 = mybir.dt.bfloat16


# ---------------------------------------------------------------- host math
def _prep_weights(Wf, beta, v, bias):
    """Reproduce the reference's weight preprocessing (tiny) on the host."""
    Wf = Wf.astype(np.float32)
    mean = np.float32(Wf.mean(dtype=np.float64))
    std = np.float32(np.sqrt(Wf.var(dtype=np.float64)))
    us = np.asarray(
        [-1.0 + i * 2.0 / (M_FILTERS - 1) for i in range(M_FILTERS)], np.float32
    )
    B = np.sign(Wf[None] - mean + us[:, None, None, None, None] * std).astype(
        np.float32
    )
    Bf = B.reshape(M_FILTERS, -1).T                      # [K, M]
    G = (Bf.T @ Bf).astype(np.float64)
    rhs = (Bf.T @ Wf.reshape(-1)).astype(np.float64)
    alpha = np.linalg.solve(G, rhs).astype(np.float32)   # [M]
    W_eff = np.einsum("m,mhwio->hwio", alpha, B).astype(np.float32)
    sum_alpha = float(alpha.sum(dtype=np.float64))
    cbias = sum_alpha * float(
        np.dot(beta.astype(np.float64), bias.astype(np.float64))
    )
    return W_eff, cbias


# kernel-variant knobs (A/B-tested via TimelineSim + HW bench)
DEFAULT_OPTS = dict(
    w2_engine="pool",      # "pool" | "vector"
    pad_only_memset=True,  # memset only the halo cells of apad
    out_dtype="bf16",      # "f32" | "bf16" (host upconverts)
    xin_bufs=6,
    scr_bufs=3,
    apad_bufs=8,   # two_phase keeps all chunk activation planes live
    ostage_bufs=3,
    taps=9,           # ablation: number of conv taps (9 = full conv)
    skip_elem=False,  # ablation: replace elementwise chain with one copy
    two_phase=True,   # emit all elementwise first, then all conv/evac/out
    out_dma="sync",   # "sync" | "scalar": which HWDGE ring stores outputs
    evac_split=0,     # groups per image evacuated on DVE instead of ACT
    merged_elem=True,  # 4-op DVE chain (q=stt(u0,c0,u2)) vs 5-op
    memset_engine="pool",  # "pool" | "vector"
    only_s1=False,    # ablation: A = b1*s1 + c0 (drops u0/u2/q DVE ops)
    no_sign=False,    # ablation: A = q (drops ACT sign + final stt)
    no_out_dma=False,  # ablation: skip the output store
    chunks=2,         # 1 | 2: row-chunked pipeline (finer overlap, less fill)
    u0_engine="vector",  # "vector" | "pool": who computes the u0 indicator
    inner_repeat=1,   # bench diagnostic: process the batch N times per iter
    delta_trick=True,  # emit A-c0 with pads=-c0; fold c0*colsum(W) into bias
    mtt_engine="vector",  # "vector" | "pool": engine for the u0+u2 add
    in_dma_split=False,  # alternate input DMAs across both HWDGE rings
    skew=None,  # software-pipeline depth: emit B_k after A_{k+skew}.
                # None -> use two_phase flag (two_phase == skew=len(work))
    img_dma=False,  # one whole-image DMA in/out shared by both chunks
    all_dve=False,  # 4-op all-DVE chain (no ACT Sign): needs |b1| not tiny
)


# Banded-pipeline variant knobs (see _emit_banded)
DEFAULT_BOPTS = dict(
    banded=True,
    band_rows=8,       # elementwise/conv band height (must divide 56)
    bskew=2,           # conv for band i emitted after elementwise of band i+bskew
    warmup=12,         # PE warmup matmuls (p-state ramp) before real work
    warmup_free=448,   # free-dim length of each warmup matmul
    in_ring="sync",    # HWDGE ring for input slabs
    out_ring="sync",   # HWDGE ring for output stores
    u0_eng="vector",   # engine computing u0 = 2b0*[x>t0]
    m_eng="vector",    # engine computing m = u0 + u2
    evac="act",        # "act" | "alt": PSUM evacuation engine(s)
    xin_bufs=7,        # divides 28 -> stable slab->buffer map per iteration
    scr_bufs=4,        # divides 28 -> stable band->buffer map
    apad_bufs=4,       # = NPER -> stable image->buffer map
    ostage_bufs=7,     # divides 28
    psum_bufs=7,       # divides 28 (8th PSUM bank free for warmup)
    in_bands_per_dma=2,  # input slab = this many bands
    out_groups_per_dma=2,  # output store batches this many conv groups
    prefetch=2,        # input slabs DMA'd before the const (wt/bias) DMAs
    const_ring="scalar",  # ring for wt/bias DMAs (off the input path)
    preload_sign=True,  # dummy Sign op at t~0 to preload the ACT table
    unroll=1,          # python-level body repeats (sim proxy for the hw loop)
)


@with_exitstack
def _emit_banded(ctx, tc, xt, wt, bv, out, consts, repeat=1, opts=None):
    """Software-pipelined band schedule.

    Per image: 7 bands of 8 rows.  Band g: DMA slab -> 4-op elementwise ->
    apad rows.  Conv group g (9 accumulating matmuls) is emitted `skew` bands
    behind the elementwise stream so PE starts ~3us into the pass instead of
    ~13us (two-phase).  Warmup matmuls on scratch data keep PE busy from t=0
    so it reaches the 2.4GHz p-state before real matmuls arrive.
    """
    bo = dict(DEFAULT_BOPTS)
    if opts:
        bo.update(opts)
    nc = tc.nc
    t0, t1, t2, two_b0, b1, two_b2, c0, cbias = consts
    odt = BF16

    BR = bo["band_rows"]
    NBAND = H // BR                       # bands (= conv groups) per image
    pv = -c0                              # delta_trick pad value

    cpool = ctx.enter_context(tc.tile_pool(name="const", bufs=1))
    xpool = ctx.enter_context(tc.tile_pool(name="xin", bufs=bo["xin_bufs"]))
    spool = ctx.enter_context(tc.tile_pool(name="scr", bufs=bo["scr_bufs"]))
    apool = ctx.enter_context(tc.tile_pool(name="apad", bufs=bo["apad_bufs"]))
    opool = ctx.enter_context(tc.tile_pool(name="ostage", bufs=bo["ostage_bufs"]))
    ppool = ctx.enter_context(
        tc.tile_pool(name="psum", bufs=bo["psum_bufs"], space=bass.MemorySpace.PSUM)
    )
    wpool = ppool
    if bo["warmup"] and bo["psum_bufs"] < 8:
        wpool = ctx.enter_context(
            tc.tile_pool(name="wpsum", bufs=1, space=bass.MemorySpace.PSUM)
        )

    _rings = {"sync": nc.sync, "scalar": nc.scalar, "pool": nc.gpsimd,
              "vector": nc.vector}
    in_eng = _rings[bo["in_ring"]]
    out_eng = _rings[bo["out_ring"]]
    const_eng = _rings[bo["const_ring"]]

    apads = {}          # n -> apad tile
    xins = {}           # (n, slab) -> xin tile

    nbp = bo["in_bands_per_dma"]

    def slab_dma(n, slab):
        srows = min(nbp * BR, H - slab * nbp * BR)
        xin = xpool.tile([C, srows, WID], F32, tag="xin", name="xin")
        in_eng.dma_start(
            xin[:], xt[:, n, slab * nbp * BR : slab * nbp * BR + srows, :]
        )
        xins[(n, slab)] = xin

    wt_sb = cpool.tile([C, 9 * C], BF16)
    const_eng.dma_start(wt_sb[:], wt[:, :])
    bias_t = cpool.tile([C, 1], F32)
    const_eng.dma_start(bias_t[:], bv[:, :])
    nt1_t = cpool.tile([C, 1], F32)
    nc.vector.memset(nt1_t[:], -t1)
    if bo["preload_sign"]:
        sdum = cpool.tile([C, 1], BF16)
        nc.scalar.activation(sdum[:], nt1_t[:], AFT.Sign, bias=nt1_t[:, 0:1])

    # --- PE warmup: scratch matmuls with no upstream deps ---------------
    if bo["warmup"]:
        wf = bo["warmup_free"]
        wscr = cpool.tile([C, wf], BF16)
        nc.gpsimd.memset(wscr[:], 0.0)
        wtag = "wpsum" if wpool is not ppool else "opsum"
        wpsum = wpool.tile([C, wf], F32, name="warm", tag=wtag)
        for i in range(bo["warmup"]):
            nc.tensor.matmul(
                wpsum[:], wscr[:, 0:C], wscr[:], start=(i == 0),
                stop=(i == bo["warmup"] - 1),
            )

    if repeat > 1:
        loop_cm = tc.For_i(0, repeat, 1, hint_engines=(mybir.EngineType.PE,))
        ctx.enter_context(loop_cm)
    u0_eng = nc.gpsimd if bo["u0_eng"] == "pool" else nc.vector
    m_eng = nc.gpsimd if bo["m_eng"] == "pool" else nc.vector

    # input slabs on the critical path lead their ring (inside the loop so
    # every hardware iteration re-DMAs them)
    nslab_img = (NBAND + nbp - 1) // nbp
    for p in range(bo["prefetch"]):
        slab_dma(p // nslab_img, p % nslab_img)

    def phase_a(n, g):
        """Slab DMA (if owning band) + elementwise band -> apad rows."""
        if g == 0:
            apad = apool.tile([C, RP, CP], BF16, tag="apad", name="apad")
            apads[n] = apad
            ms = nc.gpsimd
            ms.memset(apad[:, 0:1, :], pv)                    # top halo row
            ms.memset(apad[:, RP - 1 : RP, :], pv)            # bottom halo row
            ms.memset(apad[:, 1 : RP - 1, 0:2], pv)           # left halo cols
            ms.memset(apad[:, 1 : RP - 1, WID + 2 : CP], pv)  # right halo cols
        apad = apads[n]

        slab = g // nbp
        if (n, slab) not in xins:
            slab_dma(n, slab)
        xin = xins[(n, slab)][:, (g % nbp) * BR : (g % nbp) * BR + BR, :]

        interior = apad[:, 1 + g * BR : 1 + (g + 1) * BR, 2 : WID + 2]
        u0 = spool.tile([C, BR, WID], BF16, tag="u0", name="u0")
        u0_eng.tensor_scalar(u0[:], xin, t0, two_b0, AOT.is_gt, AOT.mult)
        u2 = spool.tile([C, BR, WID], BF16, tag="u2", name="u2")
        nc.vector.tensor_scalar(u2[:], xin, t2, two_b2, AOT.is_gt, AOT.mult)
        s1 = spool.tile([C, BR, WID], BF16, tag="s1", name="s1")
        nc.scalar.activation(s1[:], xin, AFT.Sign, bias=nt1_t[:, 0:1])
        m = spool.tile([C, BR, WID], BF16, tag="m", name="m")
        m_eng.tensor_tensor(m[:], u0[:], u2[:], AOT.add)
        nc.vector.scalar_tensor_tensor(
            interior, s1[:], b1, m[:], AOT.mult, AOT.add
        )

    ostages = {}

    def phase_b(n, g):
        """Conv group g: 9 accumulating matmuls + bias evac + store."""
        apad = apads[n]
        psum = ppool.tile([C, BR, WID], F32, name=f"ps{n}_{g}", tag="opsum")
        for tap in range(9):
            dy, dx = divmod(tap, 3)
            rhs = apad[:, g * BR + dy : g * BR + dy + BR, 1 + dx : 1 + dx + WID]
            nc.tensor.matmul(
                psum[:], wt_sb[:, tap * C : (tap + 1) * C], rhs,
                start=(tap == 0), stop=(tap == 8),
            )
        ogd = bo["out_groups_per_dma"]
        og = g // ogd                     # output-stage slot
        ng_in_slot = min(ogd, NBAND - og * ogd)
        if g % ogd == 0:
            ostages[(n, og)] = opool.tile(
                [C, ng_in_slot * BR, WID], odt, tag="ostage", name="ostage"
            )
        ostage = ostages[(n, og)]
        dst = ostage[:, (g % ogd) * BR : (g % ogd) * BR + BR, :]
        if bo["evac"] == "alt" and g % 2 == 1:
            nc.vector.tensor_scalar(
                dst, psum[:], bias_t[:, 0:1], None, AOT.add
            )
        else:
            nc.scalar.activation(
                dst, psum[:], AFT.Identity, bias=bias_t[:, 0:1], scale=1.0
            )
        if g % ogd == ng_in_slot - 1 or g == NBAND - 1:
            r0 = og * ogd * BR
            out_eng.dma_start(
                out[:, n, r0 : r0 + ng_in_slot * BR, :],
                ostages.pop((n, og))[:],
            )

    work = [(n, g) for n in range(NPER) for g in range(NBAND)]
    skew = bo["bskew"]
    for _rep in range(bo["unroll"]):
        if _rep > 0:
            apads.clear()
            xins.clear()
            for p in range(bo["prefetch"]):
                slab_dma(p // nslab_img, p % nslab_img)
        for i, (n, g) in enumerate(work):
            phase_a(n, g)
            j = i - skew
            if j >= 0:
                phase_b(*work[j])
        for j in range(max(0, len(work) - skew), len(work)):
            phase_b(*work[j])


# ---------------------------------------------------------------- device IR
@with_exitstack
def _emit(ctx, tc, xt, wt, bv, out, consts, repeat=1, opts=DEFAULT_OPTS):
    """Per-core program.  xt:[C, NPER,H,W] f32 in, wt:[C, 9*C] bf16 in,
    out:[C, NPER,H,W] f32 out.  repeat>1 wraps the body in a hardware loop
    (identical result, used only for wall-clock benchmarking)."""
    nc = tc.nc
    t0, t1, t2, two_b0, b1, two_b2, c0, cbias = consts
    odt = F32 if opts["out_dtype"] == "f32" else BF16

    cpool = ctx.enter_context(tc.tile_pool(name="const", bufs=1))
    xpool = ctx.enter_context(tc.tile_pool(name="xin", bufs=opts["xin_bufs"]))
    spool = ctx.enter_context(tc.tile_pool(name="scr", bufs=opts["scr_bufs"]))
    apool = ctx.enter_context(tc.tile_pool(name="apad", bufs=opts["apad_bufs"]))
    opool = ctx.enter_context(
        tc.tile_pool(name="ostage", bufs=opts["ostage_bufs"])
    )
    ppool = ctx.enter_context(
        tc.tile_pool(name="psum", bufs=8, space=bass.MemorySpace.PSUM)
    )

    wt_sb = cpool.tile([C, 9 * C], BF16)
    nc.sync.dma_start(wt_sb[:], wt[:, :])
    bias_t = cpool.tile([C, 1], F32)
    nc.sync.dma_start(bias_t[:], bv[:, :])
    nt1_t = cpool.tile([C, 1], F32)
    nc.vector.memset(nt1_t[:], -t1)
    # uniform-shift pad value: conv sees a uniformly shifted A everywhere
    # incl. pads; the correction (shift * colsum(W_eff)) rides in the
    # per-channel bias vector bv.
    if opts["all_dve"]:
        pv = -c0 + b1          # emitted plane = A_true + (b0+b1+b2)
    elif opts["delta_trick"]:
        pv = -c0               # emitted plane = A_true - c0
    else:
        pv = 0.0

    if repeat > 1:
        loop_cm = tc.For_i(0, repeat, 1, hint_engines=(mybir.EngineType.PE,))
        ctx.enter_context(loop_cm)

    # Row-chunk descriptors.  Each chunk owns conv groups [g0, g1), an apad
    # tile spanning global padded rows [prow0, prow1), and computes the
    # elementwise activation for image rows [irow0, irow1) (chunks overlap by
    # the conv halo).  halo_top/halo_bot say which tile edge row is zero pad.
    if opts["chunks"] == 1:
        chunk_descs = [
            dict(g0=0, g1=NGRP, prow0=0, prow1=RP, irow0=0, irow1=H,
                 halo_top=True, halo_bot=True),
        ]
    else:
        chunk_descs = [
            dict(g0=0, g1=4, prow0=0, prow1=34, irow0=0, irow1=33,
                 halo_top=True, halo_bot=False),
            dict(g0=4, g1=NGRP, prow0=32, prow1=RP, irow0=31, irow1=H,
                 halo_top=False, halo_bot=True),
        ]

    xin_cache = {}
    ostage_cache = {}

    def phase_a(n, ck):
        """DMA in + elementwise step function -> padded bf16 activation."""
        nrow = ck["irow1"] - ck["irow0"]          # elementwise rows
        trow = ck["prow1"] - ck["prow0"]          # apad tile rows
        if opts["img_dma"]:
            if n not in xin_cache:
                xfull = xpool.tile([C, H, WID], F32, tag="xin", name="xin")
                nc.sync.dma_start(xfull[:], xt[:, n, :, :])
                xin_cache[n] = xfull
            xin = xin_cache[n][:, ck["irow0"] : ck["irow1"], :]
        else:
            xin = xpool.tile([C, nrow, WID], F32, tag="xin", name="xin")
            in_eng = nc.sync
            if opts["in_dma_split"] and (ck["g0"] > 0) != (n % 2 == 1):
                in_eng = nc.scalar
            in_eng.dma_start(xin[:], xt[:, n, ck["irow0"] : ck["irow1"], :])

        apad = apool.tile([C, trow, CP], BF16, tag="apad", name="apad")
        ms = nc.gpsimd if opts["memset_engine"] == "pool" else nc.vector
        # local interior rows (everything except zero-halo edge rows)
        li0 = ck["irow0"] + 1 - ck["prow0"]
        li1 = li0 + nrow
        if opts["pad_only_memset"]:
            if ck["halo_top"]:
                ms.memset(apad[:, 0:1, :], pv)
            if ck["halo_bot"]:
                ms.memset(apad[:, trow - 1 : trow, :], pv)
            ms.memset(apad[:, li0:li1, 0:2], pv)
            ms.memset(apad[:, li0:li1, WID + 2 : CP], pv)
        else:
            ms.memset(apad[:], pv)

        interior = apad[:, li0:li1, 2 : WID + 2]
        if opts["skip_elem"]:
            nc.vector.tensor_copy(interior, xin[:])
            return apad
        if opts["only_s1"]:
            s1 = spool.tile([C, nrow, WID], BF16, tag="s1", name="s1")
            nc.scalar.activation(s1[:], xin[:], AFT.Sign, bias=nt1_t[:, 0:1])
            nc.vector.tensor_scalar(interior, s1[:], b1, c0, AOT.mult, AOT.add)
            return apad
        if opts["all_dve"]:
            # emitted = 2b0*g0 + 2b1*g1 + 2b2*g2  (constants via pad shift)
            i1 = spool.tile([C, nrow, WID], BF16, tag="u0", name="i1")
            nc.vector.tensor_scalar(
                i1[:], xin[:], t0, two_b0 / (2.0 * b1), AOT.is_gt, AOT.mult
            )
            i2 = spool.tile([C, nrow, WID], BF16, tag="u2", name="i2")
            nc.vector.tensor_scalar(
                i2[:], xin[:], t2, two_b2, AOT.is_gt, AOT.mult
            )
            i3 = spool.tile([C, nrow, WID], BF16, tag="s1", name="i3")
            nc.vector.scalar_tensor_tensor(
                i3[:], xin[:], t1, i1[:], AOT.is_gt, AOT.add
            )
            nc.vector.scalar_tensor_tensor(
                interior, i3[:], 2.0 * b1, i2[:], AOT.mult, AOT.add
            )
            return apad
        u0 = spool.tile([C, nrow, WID], BF16, tag="u0", name="u0")
        u0_eng = nc.vector if opts["u0_engine"] == "vector" else nc.gpsimd
        u0_eng.tensor_scalar(u0[:], xin[:], t0, two_b0, AOT.is_gt, AOT.mult)
        u2 = spool.tile([C, nrow, WID], BF16, tag="u2", name="u2")
        nc.vector.tensor_scalar(u2[:], xin[:], t2, two_b2, AOT.is_gt, AOT.mult)
        if opts["no_sign"]:
            nc.vector.scalar_tensor_tensor(
                interior, u0[:], c0, u2[:], AOT.add, AOT.add
            )
            return apad
        s1 = spool.tile([C, nrow, WID], BF16, tag="s1", name="s1")
        nc.scalar.activation(s1[:], xin[:], AFT.Sign, bias=nt1_t[:, 0:1])
        if opts["delta_trick"]:
            # m = u0 + u2 ; A - c0 = b1*s1 + m  (c0 folded into bias vector)
            m = spool.tile([C, nrow, WID], BF16, tag="w2", name="m")
            mtt = nc.vector if opts["mtt_engine"] == "vector" else nc.gpsimd
            mtt.tensor_tensor(m[:], u0[:], u2[:], AOT.add)
            nc.vector.scalar_tensor_tensor(
                interior, s1[:], b1, m[:], AOT.mult, AOT.add
            )
        elif opts["merged_elem"]:
            # q = u0 + c0 + u2 ; A = b1*s1 + q
            q = spool.tile([C, nrow, WID], BF16, tag="w2", name="q")
            nc.vector.scalar_tensor_tensor(
                q[:], u0[:], c0, u2[:], AOT.add, AOT.add
            )
            nc.vector.scalar_tensor_tensor(
                interior, s1[:], b1, q[:], AOT.mult, AOT.add
            )
        else:
            s1c = spool.tile([C, nrow, WID], BF16, tag="s1c", name="s1c")
            nc.vector.tensor_scalar(s1c[:], s1[:], b1, c0, AOT.mult, AOT.add)
            w2 = spool.tile([C, nrow, WID], BF16, tag="w2", name="w2")
            if opts["w2_engine"] == "pool":
                nc.gpsimd.tensor_tensor(w2[:], u0[:], u2[:], AOT.add)
            else:
                nc.vector.tensor_tensor(w2[:], u0[:], u2[:], AOT.add)
            nc.vector.scalar_tensor_tensor(
                interior, s1c[:], 0.0, w2[:], AOT.add, AOT.add
            )
        return apad

    def phase_b(n, ck, apad):
        """3x3 SAME conv (9 accumulating matmuls per 8-row group) + bias +
        store."""
        ntaps = opts["taps"]
        g0, g1 = ck["g0"], ck["g1"]
        ngrp = g1 - g0
        psums = []
        for g in range(ngrp):
            psums.append(
                ppool.tile([C, GR, WID], F32, name=f"psum_g{g}", tag="opsum")
            )
        for tap in range(ntaps):
            dy, dx = divmod(tap, 3)
            lhsT = wt_sb[:, tap * C : (tap + 1) * C]
            for g in range(g0, g1):
                r = g * GR + dy - ck["prow0"]
                rhs = apad[:, r : r + GR, 1 + dx : 1 + dx + WID]
                nc.tensor.matmul(
                    psums[g - g0][:],
                    lhsT,
                    rhs,
                    start=(tap == 0),
                    stop=(tap == ntaps - 1),
                )

        orow = ngrp * GR
        if opts["img_dma"]:
            if n not in ostage_cache:
                ostage_cache[n] = opool.tile(
                    [C, H, WID], odt, tag="ostage", name="ostage"
                )
            ofull = ostage_cache[n]
            ostage = ofull[:, g0 * GR : g0 * GR + orow, :]
        else:
            ostage = opool.tile(
                [C, orow, WID], odt, tag="ostage", name="ostage"
            )
        for g in range(ngrp):
            dst = ostage[:, g * GR : (g + 1) * GR, :]
            if g < opts["evac_split"]:
                nc.vector.tensor_scalar(
                    dst, psums[g][:], bias_t[:, 0:1], None, AOT.add
                )
            else:
                nc.scalar.activation(
                    dst,
                    psums[g][:],
                    AFT.Identity,
                    bias=bias_t[:, 0:1],
                    scale=1.0,
                )
        if not opts["no_out_dma"]:
            dma_eng = nc.sync if opts["out_dma"] == "sync" else nc.scalar
            if opts["img_dma"]:
                if g1 == NGRP:  # last chunk of the image: store whole image
                    dma_eng.dma_start(out[:, n, :, :], ostage_cache.pop(n)[:])
            else:
                dma_eng.dma_start(
                    out[:, n, g0 * GR : g0 * GR + orow, :], ostage[:]
                )

    work = [(n, ck) for n in range(NPER) for ck in chunk_descs]
    for _rep in range(opts["inner_repeat"]):
        xin_cache.clear()
        ostage_cache.clear()
        skew = opts["skew"]
        if skew is None:
            skew = len(work) if opts["two_phase"] else 0
        apads = {}
        for i, (n, ck) in enumerate(work):
            apads[i] = phase_a(n, ck)
            j = i - skew
            if j >= 0:
                phase_b(*work[j], apads.pop(j))
        for j in range(max(0, len(work) - skew), len(work)):
            phase_b(*work[j], apads.pop(j))


def build_nc(consts, repeat=1, opts=DEFAULT_OPTS):
    nc = bacc.Bacc(
        "TRN2", target_bir_lowering=False, debug=False, enable_asserts=True
    )
    odt = F32 if opts.get("out_dtype", "bf16") == "f32" else BF16
    xt = nc.dram_tensor("xt", [C, NPER, H, WID], F32, kind="ExternalInput")
    wt = nc.dram_tensor("wt", [C, 9 * C], BF16, kind="ExternalInput")
    bv = nc.dram_tensor("bv", [C, 1], F32, kind="ExternalInput")
    out = nc.dram_tensor("out", [C, NPER, H, WID], odt, kind="ExternalOutput")
    with tile.TileContext(nc) as tc:
        if opts.get("banded"):
            _emit_banded(tc, xt, wt, bv, out, consts, repeat=repeat, opts=opts)
        else:
            _emit(tc, xt, wt, bv, out, consts, repeat=repeat, opts=opts)
    nc.compile()
    return nc


_NC_CACHE = {}


def _kernel_opts():
    opts = dict(DEFAULT_OPTS)
    opts.update(DEFAULT_BOPTS)
    return opts


def _get_nc(consts):
    key = tuple(consts)
    if key not in _NC_CACHE:
        _NC_CACHE[key] = build_nc(consts, opts=_kernel_opts())
    return _NC_CACHE[key]


def make_consts(beta, v):
    t = (0.5 - v.astype(np.float64)).astype(np.float32)
    b = beta.astype(np.float32)
    return (
        float(t[0]),
        float(t[1]),
        float(t[2]),
        float(2.0 * b[0]),
        float(b[1]),
        float(2.0 * b[2]),
        float(-b[0] - b[2]),
        0.0,  # cbias patched by caller
    )


def prepare(X, W, beta, v, bias, stride):
    """Host-side prep: weight folding, sharding, channel-major transpose.
    Returns (consts, in_maps)."""
    X = np.asarray(X, dtype=np.float32)
    Wf = np.asarray(W, dtype=np.float32)
    beta = np.asarray(beta, dtype=np.float32)
    v = np.asarray(v, dtype=np.float32)
    bias = np.asarray(bias, dtype=np.float32)
    assert int(stride) == 1, "kernel hardcodes stride=1"
    assert X.shape == (NB, H, WID, C) and Wf.shape == (3, 3, C, C)

    W_eff, cbias = _prep_weights(Wf, beta, v, bias)
    consts = list(make_consts(beta, v))
    consts[7] = float(cbias)
    consts = tuple(consts)

    # weight taps, transposed layout lhsT[tap] = W_eff[dy,dx][ci,co]
    wt = np.ascontiguousarray(
        W_eff.reshape(9, C, C).transpose(1, 0, 2).reshape(C, 9 * C)
    ).astype(ml_dtypes.bfloat16)

    # per-channel output bias: constant term + uniform-shift correction.
    # pv = (emitted plane - true A), identical everywhere incl. pads, so its
    # conv contribution is pv * colsum(W) per output channel.
    c0, b1 = consts[6], consts[4]
    if DEFAULT_OPTS["all_dve"]:
        pv = -c0 + b1
    elif DEFAULT_OPTS["delta_trick"]:
        pv = -c0
    else:
        pv = 0.0
    colsum = wt.astype(np.float32).reshape(C, 9 * C).sum(axis=0)
    colsum = colsum.reshape(9, C).sum(axis=0)  # [co] over taps+ci
    bvv = (cbias - np.float32(pv) * colsum).reshape(C, 1).astype(np.float32)

    in_maps = []
    for i in range(NCORES):
        xs = X[i * NPER : (i + 1) * NPER]              # [NPER,H,W,C]
        xs = np.ascontiguousarray(np.moveaxis(xs, 3, 0))  # [C,NPER,H,W]
        in_maps.append({"xt": xs, "wt": wt, "bv": bvv})
    return consts, in_maps


def kernel(X, W, beta, v, bias, stride):
    consts, in_maps = prepare(X, W, beta, v, bias, stride)

    nc = _get_nc(consts)
    res = run_bass_kernel_spmd(nc, in_maps, core_ids=list(range(NCORES)))

    outs = []
    for i in range(NCORES):
        o = np.asarray(res.results[i]["out"], dtype=np.float32)  # [C,NPER,H,W]
        outs.append(np.moveaxis(o, 0, 3))                        # [NPER,H,W,C]
    return np.concatenate(outs, axis=0)

